# revision 1
# baseline (speedup 1.0000x reference)
"""Causal self-attention (B=4, T=2048, D=1024, H=16) on 8 trn2 NeuronCores.

Sharding: Megatron-style tensor parallel over heads (TP=2) x data parallel
over batch (DP=4). Core c handles batch c//2 and head-group c%2 (8 heads).
Each core computes its QKV projection slice, causal attention for its 8
heads, and a partial output projection; the host sums the two TP partials
per batch and adds b_proj.

All matmuls run in fp16 (fp32 PSUM accumulation); softmax runs in fp32 on
the scalar engine (exp) / DVE (reciprocal).
"""
import sys

sys.path.insert(0, "/opt/trn_rl_repo")

import numpy as np
import ml_dtypes

import concourse.bass as bass
import concourse.tile as tile
from concourse import bacc, mybir
from concourse.bass_utils import run_bass_kernel_spmd

B, T, D, H = 4, 2048, 1024, 16
HD = 64            # head dim
HL = 8             # heads per core (TP=2)
DL = HL * HD       # 512 local qkv width
KCH = D // 128     # 8 contraction chunks for QKV
TCH = T // 128     # 16 T chunks of 128
TB = T // 512      # 4 T blocks of 512
F16 = mybir.dt.float16
F32 = mybir.dt.float32
NEG = -1.0e30

_cache = {}


def _build():
    nc = bacc.Bacc("TRN2", target_bir_lowering=False, num_devices=8)

    xT = nc.dram_tensor("xT", [D, T], F16, kind="ExternalInput")
    wqk = nc.dram_tensor("wqk", [D, 2 * DL], F16, kind="ExternalInput")
    bqk = nc.dram_tensor("bqk", [128, 2 * DL // 128], F32, kind="ExternalInput")
    wv = nc.dram_tensor("wv", [D, DL], F16, kind="ExternalInput")
    bv = nc.dram_tensor("bv", [1, DL], F16, kind="ExternalInput")
    wp = nc.dram_tensor("wp", [DL, D], F16, kind="ExternalInput")
    tri = nc.dram_tensor("tri", [128, 128], F32, kind="ExternalInput")
    out = nc.dram_tensor("out", [T, D], F32, kind="ExternalOutput")

    with tile.TileContext(nc) as tc:
        with (
            tc.tile_pool(name="const", bufs=1) as const,
            tc.tile_pool(name="acts", bufs=1) as acts,
            tc.tile_pool(name="work", bufs=4) as work,
            tc.tile_pool(name="small", bufs=3) as small,
            tc.tile_pool(name="outp", bufs=3) as outp,
            tc.tile_pool(name="ps", bufs=4, space="PSUM") as ps,
            tc.tile_pool(name="psy", bufs=2, space="PSUM") as psy,
        ):
            # ---- resident inputs ----
            xT_sb = []
            wqk_sb = []
            wv_sb = []
            for k in range(KCH):
                xt = const.tile([128, T], F16, name=f"xT{k}", tag=f"xT{k}")
                nc.sync.dma_start(out=xt, in_=xT[128 * k:128 * (k + 1), :])
                xT_sb.append(xt)
                wq = const.tile([128, 2 * DL], F16, name=f"wqk{k}", tag=f"wqk{k}")
                nc.sync.dma_start(out=wq, in_=wqk[128 * k:128 * (k + 1), :])
                wqk_sb.append(wq)
                wvt = const.tile([128, DL], F16, name=f"wv{k}", tag=f"wv{k}")
                nc.sync.dma_start(out=wvt, in_=wv[128 * k:128 * (k + 1), :])
                wv_sb.append(wvt)
            wp_sb = []
            for c in range(DL // 128):
                wpt = const.tile([128, D], F16, name=f"wp{c}", tag=f"wp{c}")
                nc.sync.dma_start(out=wpt, in_=wp[128 * c:128 * (c + 1), :])
                wp_sb.append(wpt)
            bqk_sb = const.tile([128, 2 * DL // 128], F32)
            nc.sync.dma_start(out=bqk_sb, in_=bqk[:, :])
            bv_sb = const.tile([1, DL], F16)
            nc.sync.dma_start(out=bv_sb, in_=bv[:, :])
            tri_sb = const.tile([128, 128], F32)
            nc.sync.dma_start(out=tri_sb, in_=tri[:, :])
            ones_sb = const.tile([1, 128], F16)
            nc.gpsimd.memset(ones_sb, 1.0)

            # ---- persistent activations ----
            qT_sb = [acts.tile([128, T], F16, name=f"qT{c}", tag=f"qT{c}")
                     for c in range(4)]
            kT_sb = [acts.tile([128, T], F16, name=f"kT{c}", tag=f"kT{c}")
                     for c in range(4)]
            vaug = [acts.tile([128, HL * (HD + 1)], F16, name=f"va{t}",
                              tag=f"va{t}") for t in range(TCH)]
            yT_sb = [acts.tile([128, T], F16, name=f"yT{c}", tag=f"yT{c}")
                     for c in range(4)]

            # ---- phase 1: qT / kT = (w_slice)^T @ xT  [cols, T] ----
            for cc in range(2 * DL // 128):
                dst = qT_sb[cc] if cc < 4 else kT_sb[cc - 4]
                for tb in range(TB):
                    ps_s = ps.tile([128, 512], F32, name="psA", tag="psA")
                    for k in range(KCH):
                        nc.tensor.matmul(
                            ps_s,
                            wqk_sb[k][:, 128 * cc:128 * (cc + 1)],
                            xT_sb[k][:, 512 * tb:512 * (tb + 1)],
                            start=(k == 0), stop=(k == KCH - 1),
                        )
                    nc.vector.tensor_scalar_add(
                        out=dst[:, 512 * tb:512 * (tb + 1)],
                        in0=ps_s,
                        scalar1=bqk_sb[:, cc:cc + 1],
                    )

            # ---- phase 2: v (natural layout) + ones column ----
            for t in range(TCH):
                ps_v = ps.tile([128, 512], F32, name="psA", tag="psA")
                for k in range(KCH):
                    nc.tensor.matmul(
                        ps_v,
                        xT_sb[k][:, 128 * t:128 * (t + 1)],
                        wv_sb[k],
                        start=(k == 0), stop=False,
                    )
                # bias via K=1 matmul: ones^T [128,1] @ bv [1,512]
                nc.tensor.matmul(ps_v, ones_sb, bv_sb, start=False, stop=True)
                va = vaug[t]
                va3 = va.rearrange("p (h c) -> p h c", c=HD + 1)
                nc.vector.tensor_copy(
                    va3[:, :, 0:HD],
                    ps_v.rearrange("p (h d) -> p h d", d=HD),
                )
                nc.gpsimd.memset(va3[:, :, HD], 1.0)

            # ---- phase 3: attention per head ----
            for h in range(HL):
                c, poff = h // 2, 64 * (h % 2)
                qTh = qT_sb[c][poff:poff + 64, :]
                kTh = kT_sb[c][poff:poff + 64, :]
                for q0 in range(TB):
                    ntiles = 4 * q0 + 4
                    ps_y = psy.tile([HD + 1, 512], F32, name="psY", tag="psY")
                    for t in range(ntiles):
                        m = t - 4 * q0
                        lo = 128 * m if m > 0 else 0
                        w = 512 - lo
                        ps_s = ps.tile([128, 512], F32, name="psA", tag="psA")
                        nc.tensor.matmul(
                            ps_s[:, lo:512],
                            kTh[:, 128 * t:128 * (t + 1)],
                            qTh[:, 512 * q0 + lo:512 * (q0 + 1)],
                            start=True, stop=True,
                        )
                        if m >= 0:
                            nc.vector.tensor_add(
                                ps_s[:, lo:lo + 128],
                                ps_s[:, lo:lo + 128],
                                tri_sb,
                            )
                        es = work.tile([128, 512], F16, name="es", tag="es")
                        nc.scalar.activation(
                            out=es[:, lo:512],
                            in_=ps_s[:, lo:512],
                            func=mybir.ActivationFunctionType.Exp,
                        )
                        nc.tensor.matmul(
                            ps_y[:, lo:512],
                            vaug[t][:, (HD + 1) * h:(HD + 1) * (h + 1)],
                            es[:, lo:512],
                            start=(t == 0), stop=(t == ntiles - 1),
                        )
                    dn = small.tile([1, 512], F32, name="dn", tag="dn")
                    nc.vector.tensor_copy(dn, ps_y[HD:HD + 1, :])
                    rc = small.tile([1, 512], F32, name="rc", tag="rc")
                    nc.vector.reciprocal(rc, dn)
                    rcb = small.tile([64, 512], F32, name="rcb", tag="rcb")
                    nc.gpsimd.partition_broadcast(rcb, rc)
                    nc.vector.tensor_mul(
                        yT_sb[c][poff:poff + 64, 512 * q0:512 * (q0 + 1)],
                        ps_y[0:HD, :],
                        rcb,
                    )

            # ---- phase 4: partial out projection [T, D] ----
            for t in range(TCH):
                for nb in range(D // 512):
                    ps_o = ps.tile([128, 512], F32, name="psA", tag="psA")
                    for c in range(DL // 128):
                        nc.tensor.matmul(
                            ps_o,
                            yT_sb[c][:, 128 * t:128 * (t + 1)],
                            wp_sb[c][:, 512 * nb:512 * (nb + 1)],
                            start=(c == 0), stop=(c == DL // 128 - 1),
                        )
                    ob = outp.tile([128, 512], F32, name="ob", tag="ob")
                    nc.vector.tensor_copy(ob, ps_o)
                    nc.sync.dma_start(
                        out=out[128 * t:128 * (t + 1), 512 * nb:512 * (nb + 1)],
                        in_=ob,
                    )

    nc.finalize()
    return nc


def _enable_trace_hooks():
    """Inject antenv.axon_hooks + no-op artifact upload so that
    run_bass_kernel_spmd(trace=True) works under axon in this image."""
    import types
    import antenv

    if "antenv.axon_hooks" not in sys.modules:
        mod = types.ModuleType("antenv.axon_hooks")
        state = {"hook": None}
        mod.set_axon_ntff_profile_hook = lambda h: state.__setitem__("hook", h)
        mod.get_axon_ntff_profile_hook = lambda: state["hook"]
        sys.modules["antenv.axon_hooks"] = mod
        antenv.axon_hooks = mod
        from trn_agent_boot.trn_boot import _ntff_profile_via_ctypes

        mod.set_axon_ntff_profile_hook(
            _ntff_profile_via_ctypes("/opt/axon/libaxon_pjrt.so"))
    from concourse import bass_utils as bu

    bu.upload_artifacts = lambda tmpdir: str(tmpdir)


def kernel(x, w_attn, b_attn, w_proj, b_proj, _trace=False):
    x = np.asarray(x)
    w_attn = np.asarray(w_attn)
    b_attn = np.asarray(b_attn)
    w_proj = np.asarray(w_proj)
    b_proj = np.asarray(b_proj)

    if "nc" not in _cache:
        _cache["nc"] = _build()
    nc = _cache["nc"]

    scale = 1.0 / np.sqrt(HD)
    f16 = ml_dtypes.float16 if not hasattr(np, "float16") else np.float16
    tri = np.where(np.arange(128)[:, None] <= np.arange(128)[None, :],
                   np.float32(0.0), np.float32(NEG)).astype(np.float32)

    in_maps = []
    for core in range(8):
        b, hg = core // 2, core % 2
        qs = slice(hg * DL, (hg + 1) * DL)
        ks = slice(D + hg * DL, D + (hg + 1) * DL)
        vs = slice(2 * D + hg * DL, 2 * D + (hg + 1) * DL)
        wq = (w_attn[:, qs] * scale).astype(f16)
        wk = w_attn[:, ks].astype(f16)
        wqk_host = np.concatenate([wq, wk], axis=1)
        bqk_host = np.concatenate(
            [b_attn[qs] * scale, b_attn[ks]]).astype(np.float32)
        in_maps.append({
            "xT": np.ascontiguousarray(x[b].T).astype(f16),
            "wqk": np.ascontiguousarray(wqk_host),
            "bqk": np.ascontiguousarray(bqk_host.reshape(8, 128).T),
            "wv": np.ascontiguousarray(w_attn[:, vs]).astype(f16),
            "bv": np.ascontiguousarray(b_attn[vs][None, :]).astype(f16),
            "wp": np.ascontiguousarray(w_proj[hg * DL:(hg + 1) * DL, :]).astype(f16),
            "tri": tri,
        })

    kwargs = {}
    if _trace:
        _enable_trace_hooks()
        kwargs = dict(trace=True, trace_cores=[0])
    res = run_bass_kernel_spmd(nc, in_maps, core_ids=list(range(8)), **kwargs)

    outp = np.empty((B, T, D), np.float32)
    for b in range(B):
        outp[b] = res.results[2 * b]["out"] + res.results[2 * b + 1]["out"]
    outp += b_proj.astype(np.float32)

    if _trace:
        print(f"HW exec time: {res.exec_time_ns} ns")
    return outp


# revision 4
# speedup vs baseline: 1.0502x; 1.0502x over previous
"""Causal self-attention (B=4, T=2048, D=1024, H=16) on 8 trn2 NeuronCores.

Sharding: Megatron-style tensor parallel over heads (TP=2) x data parallel
over batch (DP=4). Core c handles batch c//2 and head-group c%2 (8 heads).
Each core computes its QKV projection slice, causal attention for its 8
heads, and a partial output projection; the host sums the two TP partials
per batch and adds b_proj.

All matmuls run in fp16 (fp32 PSUM accumulation); softmax runs in fp32 on
the scalar engine (exp) / DVE (reciprocal).
"""
import sys

sys.path.insert(0, "/opt/trn_rl_repo")

import numpy as np
import ml_dtypes

import concourse.bass as bass
import concourse.tile as tile
from concourse import bacc, mybir
from concourse.bass_utils import run_bass_kernel_spmd

B, T, D, H = 4, 2048, 1024, 16
HD = 64            # head dim
HL = 8             # heads per core (TP=2)
DL = HL * HD       # 512 local qkv width
KCH = D // 128     # 8 contraction chunks for QKV
TCH = T // 128     # 16 T chunks of 128
TB = T // 512      # 4 T blocks of 512
F16 = mybir.dt.float16
F32 = mybir.dt.float32
NEG = -1.0e30

_cache = {}


def _build():
    nc = bacc.Bacc("TRN2", target_bir_lowering=False, num_devices=8)

    xT = nc.dram_tensor("xT", [D, T], F16, kind="ExternalInput")
    wqk = nc.dram_tensor("wqk", [D, 2 * DL], F16, kind="ExternalInput")
    bqk = nc.dram_tensor("bqk", [128, 2 * DL // 128], F32, kind="ExternalInput")
    wv = nc.dram_tensor("wv", [D, DL], F16, kind="ExternalInput")
    bv = nc.dram_tensor("bv", [1, DL], F16, kind="ExternalInput")
    wp = nc.dram_tensor("wp", [DL, D], F16, kind="ExternalInput")
    tri = nc.dram_tensor("tri", [128, 128], F32, kind="ExternalInput")
    out = nc.dram_tensor("out", [T, D], F32, kind="ExternalOutput")

    with tile.TileContext(nc) as tc:
        with (
            tc.tile_pool(name="const", bufs=1) as const,
            tc.tile_pool(name="acts", bufs=1) as acts,
            tc.tile_pool(name="work", bufs=4) as work,
            tc.tile_pool(name="small", bufs=3) as small,
            tc.tile_pool(name="outp", bufs=3) as outp,
            tc.tile_pool(name="ps", bufs=4, space="PSUM") as ps,
            tc.tile_pool(name="psy", bufs=2, space="PSUM") as psy,
        ):
            # ---- resident inputs ----
            xT_sb = []
            wqk_sb = []
            wv_sb = []
            for k in range(KCH):
                xt = const.tile([128, T], F16, name=f"xT{k}", tag=f"xT{k}")
                nc.sync.dma_start(out=xt, in_=xT[128 * k:128 * (k + 1), :])
                xT_sb.append(xt)
                wq = const.tile([128, 2 * DL], F16, name=f"wqk{k}", tag=f"wqk{k}")
                nc.sync.dma_start(out=wq, in_=wqk[128 * k:128 * (k + 1), :])
                wqk_sb.append(wq)
                wvt = const.tile([128, DL], F16, name=f"wv{k}", tag=f"wv{k}")
                nc.sync.dma_start(out=wvt, in_=wv[128 * k:128 * (k + 1), :])
                wv_sb.append(wvt)
            wp_sb = []
            for c in range(DL // 128):
                wpt = const.tile([128, D], F16, name=f"wp{c}", tag=f"wp{c}")
                nc.sync.dma_start(out=wpt, in_=wp[128 * c:128 * (c + 1), :])
                wp_sb.append(wpt)
            bqk_sb = const.tile([128, 2 * DL // 128], F32)
            nc.sync.dma_start(out=bqk_sb, in_=bqk[:, :])
            bv_sb = const.tile([1, DL], F16)
            nc.sync.dma_start(out=bv_sb, in_=bv[:, :])
            tri_sb = const.tile([128, 128], F32)
            nc.sync.dma_start(out=tri_sb, in_=tri[:, :])
            ones_sb = const.tile([1, 128], F16)
            nc.gpsimd.memset(ones_sb, 1.0)

            # ---- persistent activations ----
            qT_sb = [acts.tile([128, T], F16, name=f"qT{c}", tag=f"qT{c}")
                     for c in range(4)]
            kT_sb = [acts.tile([128, T], F16, name=f"kT{c}", tag=f"kT{c}")
                     for c in range(4)]
            vaug = [acts.tile([128, HL * (HD + 1)], F16, name=f"va{t}",
                              tag=f"va{t}") for t in range(TCH)]
            yT_sb = [acts.tile([128, T], F16, name=f"yT{c}", tag=f"yT{c}")
                     for c in range(4)]

            # ---- phase 1: qT / kT = (w_slice)^T @ xT  [cols, T] ----
            sc_qk = nc.enter_named_scope("ph_qk", False)
            for cc in range(2 * DL // 128):
                dst = qT_sb[cc] if cc < 4 else kT_sb[cc - 4]
                for tb in range(TB):
                    ps_s = ps.tile([128, 512], F32, name="psA", tag="psA")
                    for k in range(KCH):
                        nc.tensor.matmul(
                            ps_s,
                            wqk_sb[k][:, 128 * cc:128 * (cc + 1)],
                            xT_sb[k][:, 512 * tb:512 * (tb + 1)],
                            start=(k == 0), stop=(k == KCH - 1),
                        )
                    nc.vector.tensor_scalar_add(
                        out=dst[:, 512 * tb:512 * (tb + 1)],
                        in0=ps_s,
                        scalar1=bqk_sb[:, cc:cc + 1],
                    )

            nc.leave_named_scope("ph_qk", sc_qk[0], False)
            # ---- phase 2: v (natural layout) + ones column ----
            sc_v = nc.enter_named_scope("ph_v", False)
            for t in range(TCH):
                ps_v = ps.tile([128, 512], F32, name="psA", tag="psA")
                for k in range(KCH):
                    nc.tensor.matmul(
                        ps_v,
                        xT_sb[k][:, 128 * t:128 * (t + 1)],
                        wv_sb[k],
                        start=(k == 0), stop=False,
                    )
                # bias via K=1 matmul: ones^T [128,1] @ bv [1,512]
                nc.tensor.matmul(ps_v, ones_sb, bv_sb, start=False, stop=True)
                va = vaug[t]
                va3 = va.rearrange("p (h c) -> p h c", c=HD + 1)
                nc.vector.tensor_copy(
                    va3[:, :, 0:HD],
                    ps_v.rearrange("p (h d) -> p h d", d=HD),
                )
                nc.gpsimd.memset(va3[:, :, HD], 1.0)

            nc.leave_named_scope("ph_v", sc_v[0], False)
            # ---- phase 3: attention, head pairs packed on PE row groups ----
            sc_at = nc.enter_named_scope("ph_attn", False)
            for c in range(4):
                for q0 in range(TB):
                    ntiles = 4 * q0 + 4
                    ps_ys = [psy.tile([HD + 1, 512], F32, name="psY",
                                      tag=f"psY{p}") for p in range(2)]
                    for t in range(ntiles):
                        m = t - 4 * q0
                        lo = 128 * m if m > 0 else 0
                        ess = []
                        for p in range(2):  # the two heads 2c, 2c+1
                            poff = 64 * p
                            ps_s = ps.tile([128, 512], F32, name="psA",
                                           tag="psA")
                            # lhsT base partition 0/64 -> tile_position row
                            # groups; the pair runs concurrently on the PE
                            nc.tensor.matmul(
                                ps_s[:, lo:512],
                                kT_sb[c][poff:poff + 64,
                                         128 * t:128 * (t + 1)],
                                qT_sb[c][poff:poff + 64,
                                         512 * q0 + lo:512 * (q0 + 1)],
                                start=True, stop=True,
                            )
                            if m >= 0:
                                nc.vector.tensor_add(
                                    ps_s[:, lo:lo + 128],
                                    ps_s[:, lo:lo + 128],
                                    tri_sb,
                                )
                            es = work.tile([128, 512], F16, name="es",
                                           tag="es")
                            nc.scalar.activation(
                                out=es[:, lo:512],
                                in_=ps_s[:, lo:512],
                                func=mybir.ActivationFunctionType.Exp,
                            )
                            ess.append(es)
                        for p in range(2):
                            h = 2 * c + p
                            nc.tensor.matmul(
                                ps_ys[p][:, lo:512],
                                vaug[t][:, (HD + 1) * h:(HD + 1) * (h + 1)],
                                ess[p][:, lo:512],
                                start=(t == 0), stop=(t == ntiles - 1),
                            )
                    for p in range(2):
                        poff = 64 * p
                        dn = small.tile([1, 512], F32, name="dn", tag="dn")
                        nc.vector.tensor_copy(dn, ps_ys[p][HD:HD + 1, :])
                        dnb = small.tile([64, 512], F32, name="dnb", tag="dnb")
                        nc.gpsimd.partition_broadcast(dnb, dn)
                        rcb = small.tile([64, 512], F32, name="rcb", tag="rcb")
                        nc.vector.reciprocal_approx_fast(rcb, dnb)
                        nc.vector.tensor_mul(
                            yT_sb[c][poff:poff + 64, 512 * q0:512 * (q0 + 1)],
                            ps_ys[p][0:HD, :],
                            rcb,
                        )

            nc.leave_named_scope("ph_attn", sc_at[0], False)
            # ---- phase 4: partial out projection [T, D] ----
            sc_pj = nc.enter_named_scope("ph_proj", False)
            for t in range(TCH):
                for nb in range(D // 512):
                    ps_o = ps.tile([128, 512], F32, name="psA", tag="psA")
                    for c in range(DL // 128):
                        nc.tensor.matmul(
                            ps_o,
                            yT_sb[c][:, 128 * t:128 * (t + 1)],
                            wp_sb[c][:, 512 * nb:512 * (nb + 1)],
                            start=(c == 0), stop=(c == DL // 128 - 1),
                        )
                    ob = outp.tile([128, 512], F32, name="ob", tag="ob")
                    nc.vector.tensor_copy(ob, ps_o)
                    nc.sync.dma_start(
                        out=out[128 * t:128 * (t + 1), 512 * nb:512 * (nb + 1)],
                        in_=ob,
                    )
            nc.leave_named_scope("ph_proj", sc_pj[0], False)

    nc.finalize()
    return nc


def _enable_trace_hooks():
    """Inject antenv.axon_hooks + no-op artifact upload so that
    run_bass_kernel_spmd(trace=True) works under axon in this image."""
    import types
    import antenv

    if "antenv.axon_hooks" not in sys.modules:
        mod = types.ModuleType("antenv.axon_hooks")
        state = {"hook": None}
        mod.set_axon_ntff_profile_hook = lambda h: state.__setitem__("hook", h)
        mod.get_axon_ntff_profile_hook = lambda: state["hook"]
        sys.modules["antenv.axon_hooks"] = mod
        antenv.axon_hooks = mod
        from trn_agent_boot.trn_boot import _ntff_profile_via_ctypes

        mod.set_axon_ntff_profile_hook(
            _ntff_profile_via_ctypes("/opt/axon/libaxon_pjrt.so"))
    from concourse import bass_utils as bu

    bu.upload_artifacts = lambda tmpdir: str(tmpdir)


def kernel(x, w_attn, b_attn, w_proj, b_proj, _trace=False):
    x = np.asarray(x)
    w_attn = np.asarray(w_attn)
    b_attn = np.asarray(b_attn)
    w_proj = np.asarray(w_proj)
    b_proj = np.asarray(b_proj)

    if "nc" not in _cache:
        _cache["nc"] = _build()
    nc = _cache["nc"]

    scale = 1.0 / np.sqrt(HD)
    f16 = ml_dtypes.float16 if not hasattr(np, "float16") else np.float16
    tri = np.where(np.arange(128)[:, None] <= np.arange(128)[None, :],
                   np.float32(0.0), np.float32(NEG)).astype(np.float32)

    in_maps = []
    for core in range(8):
        b, hg = core // 2, core % 2
        qs = slice(hg * DL, (hg + 1) * DL)
        ks = slice(D + hg * DL, D + (hg + 1) * DL)
        vs = slice(2 * D + hg * DL, 2 * D + (hg + 1) * DL)
        wq = (w_attn[:, qs] * scale).astype(f16)
        wk = w_attn[:, ks].astype(f16)
        wqk_host = np.concatenate([wq, wk], axis=1)
        bqk_host = np.concatenate(
            [b_attn[qs] * scale, b_attn[ks]]).astype(np.float32)
        in_maps.append({
            "xT": np.ascontiguousarray(x[b].T).astype(f16),
            "wqk": np.ascontiguousarray(wqk_host),
            "bqk": np.ascontiguousarray(bqk_host.reshape(8, 128).T),
            "wv": np.ascontiguousarray(w_attn[:, vs]).astype(f16),
            "bv": np.ascontiguousarray(b_attn[vs][None, :]).astype(f16),
            "wp": np.ascontiguousarray(w_proj[hg * DL:(hg + 1) * DL, :]).astype(f16),
            "tri": tri,
        })

    kwargs = {}
    if _trace:
        _enable_trace_hooks()
        kwargs = dict(trace=True, trace_cores=[0])
    res = run_bass_kernel_spmd(nc, in_maps, core_ids=list(range(8)), **kwargs)

    outp = np.empty((B, T, D), np.float32)
    for b in range(B):
        outp[b] = res.results[2 * b]["out"] + res.results[2 * b + 1]["out"]
    outp += b_proj.astype(np.float32)

    if _trace:
        print(f"HW exec time: {res.exec_time_ns} ns")
    return outp


# revision 7
# speedup vs baseline: 1.4365x; 1.3679x over previous
"""Causal self-attention (B=4, T=2048, D=1024, H=16) on 8 trn2 NeuronCores.

Sharding: Megatron-style tensor parallel over heads (TP=2) x data parallel
over batch (DP=4). Core c handles batch c//2 and head-group c%2 (8 heads).
Each core computes its QKV projection slice, causal attention for its 8
heads, and a partial output projection; the host sums the two TP partials
per batch and adds b_proj.

All matmuls run in fp16 (fp32 PSUM accumulation); softmax runs in fp32 on
the scalar engine (exp) / DVE (reciprocal).
"""
import sys

sys.path.insert(0, "/opt/trn_rl_repo")

import numpy as np
import ml_dtypes

import concourse.bass as bass
import concourse.tile as tile
from concourse import bacc, mybir
from concourse.bass_utils import run_bass_kernel_spmd

B, T, D, H = 4, 2048, 1024, 16
HD = 64            # head dim
HL = 8             # heads per core (TP=2)
DL = HL * HD       # 512 local qkv width
KCH = D // 128     # 8 contraction chunks for QKV
TCH = T // 128     # 16 T chunks of 128
TB = T // 512      # 4 T blocks of 512
F16 = mybir.dt.float16
F32 = mybir.dt.float32
NEG = -1.0e30

_cache = {}


def _build():
    nc = bacc.Bacc("TRN2", target_bir_lowering=False, num_devices=8)

    xT = nc.dram_tensor("xT", [D, T], F16, kind="ExternalInput")
    wqk = nc.dram_tensor("wqk", [D, 2 * DL], F16, kind="ExternalInput")
    bqk = nc.dram_tensor("bqk", [128, 2 * DL // 128], F32, kind="ExternalInput")
    wv = nc.dram_tensor("wv", [D, DL], F16, kind="ExternalInput")
    bv = nc.dram_tensor("bv", [1, DL], F16, kind="ExternalInput")
    wp = nc.dram_tensor("wp", [DL, D], F16, kind="ExternalInput")
    tri = nc.dram_tensor("tri", [128, 128], F32, kind="ExternalInput")
    out = nc.dram_tensor("out", [T, D], F32, kind="ExternalOutput")

    with tile.TileContext(nc) as tc:
        with (
            tc.tile_pool(name="const", bufs=1) as const,
            tc.tile_pool(name="acts", bufs=1) as acts,
            tc.tile_pool(name="work", bufs=4) as work,
            tc.tile_pool(name="small", bufs=3) as small,
            tc.tile_pool(name="outp", bufs=3) as outp,
            tc.tile_pool(name="ps", bufs=4, space="PSUM") as ps,
            tc.tile_pool(name="psy", bufs=2, space="PSUM") as psy,
        ):
            # ---- resident inputs ----
            xT_sb = []
            wqk_sb = []
            wv_sb = []
            for k in range(KCH):
                xt = const.tile([128, T], F16, name=f"xT{k}", tag=f"xT{k}")
                nc.sync.dma_start(out=xt, in_=xT[128 * k:128 * (k + 1), :])
                xT_sb.append(xt)
                wq = const.tile([128, 2 * DL], F16, name=f"wqk{k}", tag=f"wqk{k}")
                nc.sync.dma_start(out=wq, in_=wqk[128 * k:128 * (k + 1), :])
                wqk_sb.append(wq)
                wvt = const.tile([128, DL], F16, name=f"wv{k}", tag=f"wv{k}")
                nc.sync.dma_start(out=wvt, in_=wv[128 * k:128 * (k + 1), :])
                wv_sb.append(wvt)
            wp_sb = []
            for c in range(DL // 128):
                wpt = const.tile([128, D], F16, name=f"wp{c}", tag=f"wp{c}")
                nc.sync.dma_start(out=wpt, in_=wp[128 * c:128 * (c + 1), :])
                wp_sb.append(wpt)
            bqk_sb = const.tile([128, 2 * DL // 128], F32)
            nc.sync.dma_start(out=bqk_sb, in_=bqk[:, :])
            bv_sb = const.tile([1, DL], F16)
            nc.sync.dma_start(out=bv_sb, in_=bv[:, :])
            tri_sb = const.tile([128, 128], F32)
            nc.sync.dma_start(out=tri_sb, in_=tri[:, :])
            ones_sb = const.tile([1, 128], F16)
            nc.gpsimd.memset(ones_sb, 1.0)

            # ---- persistent activations ----
            qT_sb = [acts.tile([128, T], F16, name=f"qT{c}", tag=f"qT{c}")
                     for c in range(4)]
            # kT stored per head, zero-padded to K=128: head 2c occupies
            # partitions 0:64 (64:128 zero), head 2c+1 partitions 64:128
            # (0:64 zero).  This keeps every S matmul full-array (no
            # row-group masking, which stops the PE activity monitor from
            # registering "busy" and parks the clock at half rate).
            kT2_sb = [acts.tile([128, T], F16, name=f"kT2h{h}", tag=f"kT2h{h}")
                      for h in range(HL)]
            for h in range(HL):
                z0, z1 = (64, 128) if h % 2 == 0 else (0, 64)
                nc.gpsimd.memset(kT2_sb[h][z0:z1, :], 0.0)
            vaug = [acts.tile([128, HL * (HD + 1)], F16, name=f"va{t}",
                              tag=f"va{t}") for t in range(TCH)]
            yT_sb = [acts.tile([128, T], F16, name=f"yT{c}", tag=f"yT{c}")
                     for c in range(4)]

            # ---- phase 1: qT / kT = (w_slice)^T @ xT  [cols, T] ----
            sc_qk = nc.enter_named_scope("ph_qk", False)
            for cc in range(2 * DL // 128):
                for tb in range(TB):
                    ps_s = ps.tile([128, 512], F32, name="psA", tag="psA")
                    for k in range(KCH):
                        nc.tensor.matmul(
                            ps_s,
                            wqk_sb[k][:, 128 * cc:128 * (cc + 1)],
                            xT_sb[k][:, 512 * tb:512 * (tb + 1)],
                            start=(k == 0), stop=(k == KCH - 1),
                        )
                    tbs = slice(512 * tb, 512 * (tb + 1))
                    if cc < 4:
                        nc.vector.tensor_scalar_add(
                            out=qT_sb[cc][:, tbs],
                            in0=ps_s,
                            scalar1=bqk_sb[:, cc:cc + 1],
                        )
                    else:
                        hA = 2 * (cc - 4)
                        nc.vector.tensor_scalar_add(
                            out=kT2_sb[hA][0:64, tbs],
                            in0=ps_s[0:64, :],
                            scalar1=bqk_sb[0:64, cc:cc + 1],
                        )
                        nc.vector.tensor_scalar_add(
                            out=kT2_sb[hA + 1][64:128, tbs],
                            in0=ps_s[64:128, :],
                            scalar1=bqk_sb[64:128, cc:cc + 1],
                        )

            nc.leave_named_scope("ph_qk", sc_qk[0], False)
            # ---- phase 2: v (natural layout) + ones column ----
            sc_v = nc.enter_named_scope("ph_v", False)
            for t in range(TCH):
                ps_v = ps.tile([128, 512], F32, name="psA", tag="psA")
                for k in range(KCH):
                    nc.tensor.matmul(
                        ps_v,
                        xT_sb[k][:, 128 * t:128 * (t + 1)],
                        wv_sb[k],
                        start=(k == 0), stop=False,
                    )
                # bias via K=1 matmul: ones^T [128,1] @ bv [1,512]
                nc.tensor.matmul(ps_v, ones_sb, bv_sb, start=False, stop=True)
                va = vaug[t]
                va3 = va.rearrange("p (h c) -> p h c", c=HD + 1)
                nc.vector.tensor_copy(
                    va3[:, :, 0:HD],
                    ps_v.rearrange("p (h d) -> p h d", d=HD),
                )
                nc.gpsimd.memset(va3[:, :, HD], 1.0)

            nc.leave_named_scope("ph_v", sc_v[0], False)
            # ---- phase 3: attention, head pairs packed on PE row groups ----
            sc_at = nc.enter_named_scope("ph_attn", False)
            for c in range(4):
                for q0 in range(TB):
                    ntiles = 4 * q0 + 4
                    ps_ys = [psy.tile([HD + 1, 512], F32, name="psY",
                                      tag=f"psY{p}") for p in range(2)]
                    for t in range(ntiles):
                        m = t - 4 * q0
                        lo = 128 * m if m > 0 else 0
                        ess = []
                        for p in range(2):  # the two heads 2c, 2c+1
                            ps_s = ps.tile([128, 512], F32, name="psA",
                                           tag="psA")
                            # full-K matmul: zero-padded kT kills the other
                            # head's rows of qT
                            nc.tensor.matmul(
                                ps_s[:, lo:512],
                                kT2_sb[2 * c + p][:, 128 * t:128 * (t + 1)],
                                qT_sb[c][:, 512 * q0 + lo:512 * (q0 + 1)],
                                start=True, stop=True,
                            )
                            if m >= 0:
                                nc.vector.tensor_add(
                                    ps_s[:, lo:lo + 128],
                                    ps_s[:, lo:lo + 128],
                                    tri_sb,
                                )
                            es = work.tile([128, 512], F16, name="es",
                                           tag="es")
                            nc.scalar.activation(
                                out=es[:, lo:512],
                                in_=ps_s[:, lo:512],
                                func=mybir.ActivationFunctionType.Exp,
                            )
                            ess.append(es)
                        for p in range(2):
                            h = 2 * c + p
                            nc.tensor.matmul(
                                ps_ys[p][:, lo:512],
                                vaug[t][:, (HD + 1) * h:(HD + 1) * (h + 1)],
                                ess[p][:, lo:512],
                                start=(t == 0), stop=(t == ntiles - 1),
                            )
                    for p in range(2):
                        poff = 64 * p
                        dn = small.tile([1, 512], F32, name="dn", tag="dn")
                        nc.vector.tensor_copy(dn, ps_ys[p][HD:HD + 1, :])
                        dnb = small.tile([64, 512], F32, name="dnb", tag="dnb")
                        nc.gpsimd.partition_broadcast(dnb, dn)
                        rcb = small.tile([64, 512], F32, name="rcb", tag="rcb")
                        nc.vector.reciprocal_approx_fast(rcb, dnb)
                        nc.vector.tensor_mul(
                            yT_sb[c][poff:poff + 64, 512 * q0:512 * (q0 + 1)],
                            ps_ys[p][0:HD, :],
                            rcb,
                        )

            nc.leave_named_scope("ph_attn", sc_at[0], False)
            # ---- phase 4: partial out projection [T, D] ----
            sc_pj = nc.enter_named_scope("ph_proj", False)
            for t in range(TCH):
                for nb in range(D // 512):
                    ps_o = ps.tile([128, 512], F32, name="psA", tag="psA")
                    for c in range(DL // 128):
                        nc.tensor.matmul(
                            ps_o,
                            yT_sb[c][:, 128 * t:128 * (t + 1)],
                            wp_sb[c][:, 512 * nb:512 * (nb + 1)],
                            start=(c == 0), stop=(c == DL // 128 - 1),
                        )
                    ob = outp.tile([128, 512], F32, name="ob", tag="ob")
                    nc.vector.tensor_copy(ob, ps_o)
                    nc.sync.dma_start(
                        out=out[128 * t:128 * (t + 1), 512 * nb:512 * (nb + 1)],
                        in_=ob,
                    )
            nc.leave_named_scope("ph_proj", sc_pj[0], False)

    nc.finalize()
    return nc


def _enable_trace_hooks():
    """Inject antenv.axon_hooks + no-op artifact upload so that
    run_bass_kernel_spmd(trace=True) works under axon in this image."""
    import types
    import antenv

    if "antenv.axon_hooks" not in sys.modules:
        mod = types.ModuleType("antenv.axon_hooks")
        state = {"hook": None}
        mod.set_axon_ntff_profile_hook = lambda h: state.__setitem__("hook", h)
        mod.get_axon_ntff_profile_hook = lambda: state["hook"]
        sys.modules["antenv.axon_hooks"] = mod
        antenv.axon_hooks = mod
        from trn_agent_boot.trn_boot import _ntff_profile_via_ctypes

        mod.set_axon_ntff_profile_hook(
            _ntff_profile_via_ctypes("/opt/axon/libaxon_pjrt.so"))
    from concourse import bass_utils as bu

    bu.upload_artifacts = lambda tmpdir: str(tmpdir)


def kernel(x, w_attn, b_attn, w_proj, b_proj, _trace=False):
    x = np.asarray(x)
    w_attn = np.asarray(w_attn)
    b_attn = np.asarray(b_attn)
    w_proj = np.asarray(w_proj)
    b_proj = np.asarray(b_proj)

    if "nc" not in _cache:
        _cache["nc"] = _build()
    nc = _cache["nc"]

    scale = 1.0 / np.sqrt(HD)
    f16 = ml_dtypes.float16 if not hasattr(np, "float16") else np.float16
    tri = np.where(np.arange(128)[:, None] <= np.arange(128)[None, :],
                   np.float32(0.0), np.float32(NEG)).astype(np.float32)

    in_maps = []
    for core in range(8):
        b, hg = core // 2, core % 2
        qs = slice(hg * DL, (hg + 1) * DL)
        ks = slice(D + hg * DL, D + (hg + 1) * DL)
        vs = slice(2 * D + hg * DL, 2 * D + (hg + 1) * DL)
        wq = (w_attn[:, qs] * scale).astype(f16)
        wk = w_attn[:, ks].astype(f16)
        wqk_host = np.concatenate([wq, wk], axis=1)
        bqk_host = np.concatenate(
            [b_attn[qs] * scale, b_attn[ks]]).astype(np.float32)
        in_maps.append({
            "xT": np.ascontiguousarray(x[b].T).astype(f16),
            "wqk": np.ascontiguousarray(wqk_host),
            "bqk": np.ascontiguousarray(bqk_host.reshape(8, 128).T),
            "wv": np.ascontiguousarray(w_attn[:, vs]).astype(f16),
            "bv": np.ascontiguousarray(b_attn[vs][None, :]).astype(f16),
            "wp": np.ascontiguousarray(w_proj[hg * DL:(hg + 1) * DL, :]).astype(f16),
            "tri": tri,
        })

    kwargs = {}
    if _trace:
        _enable_trace_hooks()
        kwargs = dict(trace=True, trace_cores=[0])
    res = run_bass_kernel_spmd(nc, in_maps, core_ids=list(range(8)), **kwargs)

    outp = np.empty((B, T, D), np.float32)
    for b in range(B):
        outp[b] = res.results[2 * b]["out"] + res.results[2 * b + 1]["out"]
    outp += b_proj.astype(np.float32)

    if _trace:
        print(f"HW exec time: {res.exec_time_ns} ns")
    return outp


# revision 11
# speedup vs baseline: 1.4430x; 1.0045x over previous
"""Causal self-attention (B=4, T=2048, D=1024, H=16) on 8 trn2 NeuronCores.

Sharding: Megatron-style tensor parallel over heads (TP=2) x data parallel
over batch (DP=4). Core c handles batch c//2 and head-group c%2 (8 heads).
Each core computes its QKV projection slice, causal attention for its 8
heads, and a partial output projection; the host sums the two TP partials
per batch and adds b_proj.

All matmuls run in fp16 (fp32 PSUM accumulation); softmax runs in fp32 on
the scalar engine (exp) / DVE (reciprocal).
"""
import sys

sys.path.insert(0, "/opt/trn_rl_repo")

import numpy as np
import ml_dtypes

import concourse.bass as bass
import concourse.tile as tile
from concourse import bacc, mybir
from concourse.bass_utils import run_bass_kernel_spmd

B, T, D, H = 4, 2048, 1024, 16
HD = 64            # head dim
HL = 8             # heads per core (TP=2)
DL = HL * HD       # 512 local qkv width
KCH = D // 128     # 8 contraction chunks for QKV
TCH = T // 128     # 16 T chunks of 128
TB = T // 512      # 4 T blocks of 512
F16 = mybir.dt.float16
F32 = mybir.dt.float32
NEG = -1.0e30

_cache = {}


def _build():
    nc = bacc.Bacc("TRN2", target_bir_lowering=False, num_devices=8)

    xT = nc.dram_tensor("xT", [D, T], F16, kind="ExternalInput")
    wqk = nc.dram_tensor("wqk", [D, 2 * DL], F16, kind="ExternalInput")
    bqk = nc.dram_tensor("bqk", [128, 2 * DL // 128], F32, kind="ExternalInput")
    wv = nc.dram_tensor("wv", [D, DL], F16, kind="ExternalInput")
    bv = nc.dram_tensor("bv", [1, DL], F16, kind="ExternalInput")
    wp = nc.dram_tensor("wp", [DL, D], F16, kind="ExternalInput")
    tri = nc.dram_tensor("tri", [128, 128], F32, kind="ExternalInput")
    out = nc.dram_tensor("out", [T, D], F32, kind="ExternalOutput")

    with tile.TileContext(nc) as tc:
        with (
            tc.tile_pool(name="const", bufs=1) as const,
            tc.tile_pool(name="acts", bufs=1) as acts,
            tc.tile_pool(name="work", bufs=4) as work,
            tc.tile_pool(name="small", bufs=3) as small,
            tc.tile_pool(name="outp", bufs=3) as outp,
            tc.tile_pool(name="ps", bufs=7, space="PSUM") as ps,
        ):
            # ---- resident inputs ----
            xT_sb = []
            wqk_sb = []
            wv_sb = []
            for k in range(KCH):
                xt = const.tile([128, T], F16, name=f"xT{k}", tag=f"xT{k}")
                nc.sync.dma_start(out=xt, in_=xT[128 * k:128 * (k + 1), :])
                xT_sb.append(xt)
                wq = const.tile([128, 2 * DL], F16, name=f"wqk{k}", tag=f"wqk{k}")
                nc.sync.dma_start(out=wq, in_=wqk[128 * k:128 * (k + 1), :])
                wqk_sb.append(wq)
                wvt = const.tile([128, DL], F16, name=f"wv{k}", tag=f"wv{k}")
                nc.gpsimd.dma_start(out=wvt, in_=wv[128 * k:128 * (k + 1), :])
                wv_sb.append(wvt)
            wp_sb = []
            for c in range(DL // 128):
                wpt = const.tile([128, D], F16, name=f"wp{c}", tag=f"wp{c}")
                nc.gpsimd.dma_start(out=wpt, in_=wp[128 * c:128 * (c + 1), :])
                wp_sb.append(wpt)
            bqk_sb = const.tile([128, 2 * DL // 128], F32)
            nc.gpsimd.dma_start(out=bqk_sb, in_=bqk[:, :])
            bv_sb = const.tile([1, DL], F16)
            nc.gpsimd.dma_start(out=bv_sb, in_=bv[:, :])
            tri_sb = const.tile([128, 128], F32)
            nc.gpsimd.dma_start(out=tri_sb, in_=tri[:, :])
            ones_sb = const.tile([1, 128], F16)
            nc.gpsimd.memset(ones_sb, 1.0)

            # ---- persistent activations ----
            qT_sb = [acts.tile([128, T], F16, name=f"qT{c}", tag=f"qT{c}")
                     for c in range(4)]
            # kT stored per head, zero-padded to K=128: head 2c occupies
            # partitions 0:64 (64:128 zero), head 2c+1 partitions 64:128
            # (0:64 zero).  This keeps every S matmul full-array (no
            # row-group masking, which stops the PE activity monitor from
            # registering "busy" and parks the clock at half rate).
            kT2_sb = [acts.tile([128, T], F16, name=f"kT2h{h}", tag=f"kT2h{h}")
                      for h in range(HL)]
            for h in range(HL):
                z0, z1 = (64, 128) if h % 2 == 0 else (0, 64)
                nc.gpsimd.memset(kT2_sb[h][z0:z1, :], 0.0)
            vaug = [acts.tile([128, HL * (HD + 1)], F16, name=f"va{t}",
                              tag=f"va{t}") for t in range(TCH)]
            yT_sb = [acts.tile([128, T], F16, name=f"yT{c}", tag=f"yT{c}")
                     for c in range(4)]

            # ---- phase 1: qT / kT = (w_slice)^T @ xT  [cols, T] ----
            sc_qk = nc.enter_named_scope("ph_qk", False)
            for cc in range(2 * DL // 128):
                for tb in range(TB):
                    ps_s = ps.tile([128, 512], F32, name="psA", tag="psA")
                    for k in range(KCH):
                        nc.tensor.matmul(
                            ps_s,
                            wqk_sb[k][:, 128 * cc:128 * (cc + 1)],
                            xT_sb[k][:, 512 * tb:512 * (tb + 1)],
                            start=(k == 0), stop=(k == KCH - 1),
                        )
                    tbs = slice(512 * tb, 512 * (tb + 1))
                    if cc < 4:
                        nc.vector.tensor_scalar_add(
                            out=qT_sb[cc][:, tbs],
                            in0=ps_s,
                            scalar1=bqk_sb[:, cc:cc + 1],
                        )
                    else:
                        hA = 2 * (cc - 4)
                        nc.vector.tensor_scalar_add(
                            out=kT2_sb[hA][0:64, tbs],
                            in0=ps_s[0:64, :],
                            scalar1=bqk_sb[0:64, cc:cc + 1],
                        )
                        nc.vector.tensor_scalar_add(
                            out=kT2_sb[hA + 1][64:128, tbs],
                            in0=ps_s[64:128, :],
                            scalar1=bqk_sb[64:128, cc:cc + 1],
                        )

            nc.leave_named_scope("ph_qk", sc_qk[0], False)
            # ---- phase 2: v (natural layout) + ones column ----
            sc_v = nc.enter_named_scope("ph_v", False)
            for t in range(TCH):
                ps_v = ps.tile([128, 512], F32, name="psA", tag="psA")
                for k in range(KCH):
                    nc.tensor.matmul(
                        ps_v,
                        xT_sb[k][:, 128 * t:128 * (t + 1)],
                        wv_sb[k],
                        start=(k == 0), stop=False,
                    )
                # bias via K=1 matmul: ones^T [128,1] @ bv [1,512]
                nc.tensor.matmul(ps_v, ones_sb, bv_sb, start=False, stop=True)
                va = vaug[t]
                va3 = va.rearrange("p (h c) -> p h c", c=HD + 1)
                nc.vector.tensor_copy(
                    va3[:, :, 0:HD],
                    ps_v.rearrange("p (h d) -> p h d", d=HD),
                )
                nc.gpsimd.memset(va3[:, :, HD], 1.0)

            nc.leave_named_scope("ph_v", sc_v[0], False)
            # ---- phase 3: attention, head pairs packed on PE row groups ----
            sc_at = nc.enter_named_scope("ph_attn", False)
            for c in range(4):
                for q0 in range(TB):
                    ntiles = 4 * q0 + 4
                    ps_ys = [ps.tile([128, 512], F32, name="psY", tag="psA")
                             for p in range(2)]
                    for t in range(ntiles):
                        m = t - 4 * q0
                        lo = 128 * m if m > 0 else 0
                        ess = []
                        for p in range(2):  # the two heads 2c, 2c+1
                            ps_s = ps.tile([128, 512], F32, name="psA",
                                           tag="psA")
                            # full-K matmul: zero-padded kT kills the other
                            # head's rows of qT
                            nc.tensor.matmul(
                                ps_s[:, lo:512],
                                kT2_sb[2 * c + p][:, 128 * t:128 * (t + 1)],
                                qT_sb[c][:, 512 * q0 + lo:512 * (q0 + 1)],
                                start=True, stop=True,
                            )
                            if m >= 0:
                                nc.vector.tensor_add(
                                    ps_s[:, lo:lo + 128],
                                    ps_s[:, lo:lo + 128],
                                    tri_sb,
                                )
                            es = work.tile([128, 512], F16, name="es",
                                           tag="es")
                            nc.scalar.activation(
                                out=es[:, lo:512],
                                in_=ps_s[:, lo:512],
                                func=mybir.ActivationFunctionType.Exp,
                            )
                            ess.append(es)
                        for p in range(2):
                            h = 2 * c + p
                            nc.tensor.matmul(
                                ps_ys[p][0:HD + 1, lo:512],
                                vaug[t][:, (HD + 1) * h:(HD + 1) * (h + 1)],
                                ess[p][:, lo:512],
                                start=(t == 0), stop=(t == ntiles - 1),
                            )
                    for p in range(2):
                        poff = 64 * p
                        dn = small.tile([1, 512], F32, name="dn", tag="dn")
                        nc.vector.tensor_copy(dn, ps_ys[p][HD:HD + 1, :])
                        dnb = small.tile([64, 512], F32, name="dnb", tag="dnb")
                        nc.gpsimd.partition_broadcast(dnb, dn)
                        rcb = small.tile([64, 512], F32, name="rcb", tag="rcb")
                        nc.vector.reciprocal_approx_fast(rcb, dnb)
                        nc.vector.tensor_mul(
                            yT_sb[c][poff:poff + 64, 512 * q0:512 * (q0 + 1)],
                            ps_ys[p][0:HD, :],
                            rcb,
                        )

            nc.leave_named_scope("ph_attn", sc_at[0], False)
            # ---- phase 4: partial out projection [T, D] ----
            sc_pj = nc.enter_named_scope("ph_proj", False)
            for t in range(TCH):
                for nb in range(D // 512):
                    ps_o = ps.tile([128, 512], F32, name="psA", tag="psA")
                    for c in range(DL // 128):
                        nc.tensor.matmul(
                            ps_o,
                            yT_sb[c][:, 128 * t:128 * (t + 1)],
                            wp_sb[c][:, 512 * nb:512 * (nb + 1)],
                            start=(c == 0), stop=(c == DL // 128 - 1),
                        )
                    ob = outp.tile([128, 512], F32, name="ob", tag="ob")
                    nc.vector.tensor_copy(ob, ps_o)
                    nc.sync.dma_start(
                        out=out[128 * t:128 * (t + 1), 512 * nb:512 * (nb + 1)],
                        in_=ob,
                    )
            nc.leave_named_scope("ph_proj", sc_pj[0], False)

    nc.finalize()
    return nc


def _enable_trace_hooks():
    """Inject antenv.axon_hooks + no-op artifact upload so that
    run_bass_kernel_spmd(trace=True) works under axon in this image."""
    import types
    import antenv

    if "antenv.axon_hooks" not in sys.modules:
        mod = types.ModuleType("antenv.axon_hooks")
        state = {"hook": None}
        mod.set_axon_ntff_profile_hook = lambda h: state.__setitem__("hook", h)
        mod.get_axon_ntff_profile_hook = lambda: state["hook"]
        sys.modules["antenv.axon_hooks"] = mod
        antenv.axon_hooks = mod
        from trn_agent_boot.trn_boot import _ntff_profile_via_ctypes

        mod.set_axon_ntff_profile_hook(
            _ntff_profile_via_ctypes("/opt/axon/libaxon_pjrt.so"))
    from concourse import bass_utils as bu

    bu.upload_artifacts = lambda tmpdir: str(tmpdir)


def kernel(x, w_attn, b_attn, w_proj, b_proj, _trace=False):
    x = np.asarray(x)
    w_attn = np.asarray(w_attn)
    b_attn = np.asarray(b_attn)
    w_proj = np.asarray(w_proj)
    b_proj = np.asarray(b_proj)

    if "nc" not in _cache:
        _cache["nc"] = _build()
    nc = _cache["nc"]

    scale = 1.0 / np.sqrt(HD)
    f16 = ml_dtypes.float16 if not hasattr(np, "float16") else np.float16
    tri = np.where(np.arange(128)[:, None] <= np.arange(128)[None, :],
                   np.float32(0.0), np.float32(NEG)).astype(np.float32)

    in_maps = []
    for core in range(8):
        b, hg = core // 2, core % 2
        qs = slice(hg * DL, (hg + 1) * DL)
        ks = slice(D + hg * DL, D + (hg + 1) * DL)
        vs = slice(2 * D + hg * DL, 2 * D + (hg + 1) * DL)
        wq = (w_attn[:, qs] * scale).astype(f16)
        wk = w_attn[:, ks].astype(f16)
        wqk_host = np.concatenate([wq, wk], axis=1)
        bqk_host = np.concatenate(
            [b_attn[qs] * scale, b_attn[ks]]).astype(np.float32)
        in_maps.append({
            "xT": np.ascontiguousarray(x[b].T).astype(f16),
            "wqk": np.ascontiguousarray(wqk_host),
            "bqk": np.ascontiguousarray(bqk_host.reshape(8, 128).T),
            "wv": np.ascontiguousarray(w_attn[:, vs]).astype(f16),
            "bv": np.ascontiguousarray(b_attn[vs][None, :]).astype(f16),
            "wp": np.ascontiguousarray(w_proj[hg * DL:(hg + 1) * DL, :]).astype(f16),
            "tri": tri,
        })

    kwargs = {}
    if _trace:
        _enable_trace_hooks()
        kwargs = dict(trace=True, trace_cores=[0])
    res = run_bass_kernel_spmd(nc, in_maps, core_ids=list(range(8)), **kwargs)

    outp = np.empty((B, T, D), np.float32)
    for b in range(B):
        outp[b] = res.results[2 * b]["out"] + res.results[2 * b + 1]["out"]
    outp += b_proj.astype(np.float32)

    if _trace:
        print(f"HW exec time: {res.exec_time_ns} ns")
    return outp
